# revision 43
# baseline (speedup 1.0000x reference)
"""Trainium2 Bass kernel for AttentionAggregate (GAT-style neighbor aggregation).

Reference computation (per node n, neighbors k=0..K-1):
    pt = target @ W.T + b                      # [N, D]
    pm = middle @ W.T + b                      # [N, K, D]
    score = leaky_relu((pt[:,None,:] + pm) @ a_w.T + a_b)
    coef  = softmax(score, axis=K)
    out   = sum_k coef * middle                # [N, D]

Key algebraic simplification: the W-projection only enters through the dot
with a_w, so with u = a_w @ W (a single D-vector) and c = 2*(a_w.b) + a_b:
    score[n,k] = target[n].u + middle[n,k].u + c
This removes all large matmuls; the kernel is a memory-bound pass over
`middle` (512 MiB) with per-node softmax weighting.

Sharding: data-parallel over nodes. N=16384 nodes split across 8 cores
(2048 nodes each); W/b/a_w/a_b replicated; no cross-core communication.

Engine assignment per 128-node tile (node on partition), sized against the
~11 us/tile DMA slot (4 MiB HBM read at ~330 GB/s):
  Pool: issues all input DMAs as casting SWDGE transfers — the DMA converts
        f32 (HBM) -> fp16 (SBUF) in flight, so no on-chip convert pass is
        needed. fp16 (10 mantissa bits) keeps rounding ~4x below bf16; all
        values here are O(+-10), well inside fp16 range. Tile 0's cast is
        issued first so setup overlaps it; small setup DMAs ride the same
        queue (the sync HWDGE queue gets starved by the SWDGE stream).
  DVE:  one big fp16 multiply m2 = mid*u (2x 16-bit perf mode; TENSOR_REDUCE
        has no 2x mode, so the 256->16 reduction runs as four halving
        tensor_adds at 2x before one small 1x reduce), the exp-max, the
        one-op diag build, and reciprocal.
  ACT:  two exps (leaky-relu folded in via
        exp(leaky(x)) = max(exp(x), exp(0.01x)), biases precomputed per
        node), e pair-packing with fused denominator accumulation, and the
        PSUM evacuation scaled by 1/den.
  PE:   32 accumulating fp16 matmuls diag(e_k) @ mid[:,k,:] per tile. All
        32 diag blocks come from ONE DVE op: identity-replica times e
        broadcast along the block column; e is pre-packed into pairs
        [P, K, 2] so the broadcast AP keeps a packed innermost dim and the
        op runs in 2x mode.
  Sync: output DMAs on the SP HWDGE queue.

The loop is software-pipelined one stage deep: tile t's softmax/diag/PE
work is emitted after tile t+1's phase-1, so the ACT exp round trip never
bubbles the DVE. Softmax runs without max-subtraction: scores are O(+-8),
exp stays well inside fp16/f32 range.
"""

from contextlib import ExitStack

import numpy as np

import concourse.bass as bass
import concourse.tile as tile
from concourse import mybir
from concourse.bass_utils import run_bass_kernel_spmd

N_CORES = 8
N, K, D = 16384, 32, 256
NS = N // N_CORES  # nodes per core
P = 128
F32 = mybir.dt.float32
F16 = mybir.dt.float16
ALU = mybir.AluOpType
AF = mybir.ActivationFunctionType
AX = mybir.AxisListType
NEG_SLOPE = 0.01


def emit_kernel(tc, out, tgt, mid, u_pre, c_pre, ident, ns):
    nc = tc.nc
    nt = ns // P  # node tiles per core
    with ExitStack() as ctx:
        singles = ctx.enter_context(tc.tile_pool(name="singles", bufs=1))
        mids = ctx.enter_context(tc.tile_pool(name="mids", bufs=5))
        small = ctx.enter_context(tc.tile_pool(name="small", bufs=6))
        scr = ctx.enter_context(tc.tile_pool(name="scr", bufs=1))
        dgs = ctx.enter_context(tc.tile_pool(name="dgs", bufs=4))
        opsum = ctx.enter_context(tc.tile_pool(name="opsum", bufs=2, space="PSUM"))
        outs = ctx.enter_context(tc.tile_pool(name="outs", bufs=3))

        # ---- setup ----
        # u = a_w @ W and c = 2*(a_w.b) + a_b are precomputed on the HOST
        # (tiny numpy) and arrive as u_pre (fp16) / c_pre (f32); ident is
        # host-built fp16. This deletes the whole on-device u-chain, whose
        # W-load was starved ~15us behind tile 0's cast stream.
        # Queue strategy: tile 0's cast is the Pool/SWDGE queue's FIRST
        # issue (each Pool dma_start costs ~0.65us of issue time); tg_all
        # follows it. The tiny no-cast setup loads ride the sync HWDGE
        # queue, which only starves AFTER the cast stream ramps (~9.6us).
        # tile 0 arrives as two k-halves so the first half's multiply can
        # overlap the second half's transfer (DVE op cost scales with free
        # size, so the split halves the first compute op too)
        m0 = mids.tile([P, K, D], F16, tag="mid")
        KH = K // 2
        nc.gpsimd.dma_start(m0[:, 0:KH, :], mid[0:P, 0:KH, :])
        nc.gpsimd.dma_start(m0[:, KH:K, :], mid[0:P, KH:K, :])
        # target, all tiles at once: tg_all[p, t, d] = tgt[t*128+p, d]
        # (cast to fp16 in the DMA; feeds a fp16 2x dot against u)
        tg_all = singles.tile([P, nt, D], F16)
        nc.gpsimd.dma_start(tg_all, tgt.rearrange("(t p) d -> p t d", p=P))

        id_h = singles.tile([P, P], F16)
        nc.sync.dma_start(id_h, ident)
        u_h = singles.tile([P, D], F16)
        nc.sync.dma_start(u_h, u_pre.partition_broadcast(P))
        c_b = singles.tile([P, 1], F32)
        nc.sync.dma_start(c_b, c_pre.partition_broadcast(P))

        # per-node constants (emitted AFTER tile 0's phase-1 so the DVE can
        # start on tile 0 the moment its data lands): stc_c = target.u + c,
        # and 0.01x it. The leaky-relu folds into two ACT exps via
        #   exp(leaky(x)) = max(exp(x), exp(0.01x));
        # with x = s + stc_c, the second exp is exp(0.01*s + 0.01*stc_c).
        stc = singles.tile([P, nt], F16)
        stc_c = singles.tile([P, nt], F32)
        stc_c001 = singles.tile([P, nt], F32)
        tg_scr = scr.tile([P, nt, D], F16, tag="tg_scr")
        tg_tr = scr.tile([P, nt, D // 2], F16, tag="tg_tr")

        def emit_stc():
            nc.vector.tensor_mul(
                tg_scr, tg_all, u_h.unsqueeze(1).broadcast_to([P, nt, D])
            )
            nc.vector.tensor_add(
                tg_tr, tg_scr[:, :, 0 : D // 2], tg_scr[:, :, D // 2 : D]
            )
            with nc.allow_low_precision("fp16 scores, tolerance is 2e-2"):
                nc.vector.reduce_sum(stc, tg_tr, AX.X)
            nc.vector.tensor_scalar_add(stc_c, stc, c_b)
            nc.vector.tensor_scalar_mul(stc_c001, stc_c, 0.01)

        m2h_scr = scr.tile([P, K, D], F16, tag="m2h_scr")
        tr_a = scr.tile([P, K, D // 2], F16, tag="tr_a")
        tr_b = scr.tile([P, K, D // 4], F16, tag="tr_b")
        tr_c = scr.tile([P, K, D // 8], F16, tag="tr_c")
        tr_d = scr.tile([P, K, D // 16], F16, tag="tr_d")
        u_h_bc = u_h.unsqueeze(1).broadcast_to([P, K, D])

        # identity replicated K times along free: idK[p, k, q] = (p == q)
        # (copied after tile 0's phase-1 — only needed by the first dgall)
        idK = singles.tile([P, K, P], F16)

        def emit_idk():
            nc.vector.tensor_copy(
                idK, id_h.unsqueeze(1).broadcast_to([P, K, P])
            )

        def phase1(t):
            """Stream in tile t (cast DMA) and compute raw scores s[:, k]."""
            if t == 0:
                m = m0
            else:
                m = mids.tile([P, K, D], F16, tag="mid")
                nc.gpsimd.dma_start(m, mid[t * P : (t + 1) * P, :, :])
            s = small.tile([P, K], F16, tag="s")
            if t == 0:
                uhb_h = u_h.unsqueeze(1).broadcast_to([P, KH, D])
                nc.vector.tensor_mul(m2h_scr[:, 0:KH, :], m[:, 0:KH, :], uhb_h)
                nc.vector.tensor_mul(m2h_scr[:, KH:K, :], m[:, KH:K, :], uhb_h)
            else:
                nc.vector.tensor_mul(m2h_scr, m, u_h_bc)
            h = D // 2
            nc.vector.tensor_add(tr_a, m2h_scr[:, :, 0:h], m2h_scr[:, :, h:D])
            nc.vector.tensor_add(
                tr_b, tr_a[:, :, 0 : h // 2], tr_a[:, :, h // 2 : h]
            )
            nc.vector.tensor_add(
                tr_c, tr_b[:, :, 0 : h // 4], tr_b[:, :, h // 4 : h // 2]
            )
            nc.vector.tensor_add(
                tr_d, tr_c[:, :, 0 : h // 8], tr_c[:, :, h // 8 : h // 4]
            )
            with nc.allow_low_precision("fp16 scores, tolerance is 2e-2"):
                nc.vector.reduce_sum(s, tr_d, AX.X)
            return m, s

        def finish(t, m, s):
            """Softmax (exp-max leaky), diag build, PE aggregation, out."""
            # e = exp(leaky(s + stc_c)) = max(exp(s + A), exp(0.01(s + A)))
            e1 = small.tile([P, K], F16, tag="e1")
            e2x = small.tile([P, K], F16, tag="e2x")
            nc.scalar.activation(
                e1, s, AF.Exp, bias=stc_c[:, t : t + 1], scale=1.0
            )
            nc.scalar.activation(
                e2x, s, AF.Exp, bias=stc_c001[:, t : t + 1], scale=0.01
            )
            e = small.tile([P, K], F32, tag="e")
            nc.vector.tensor_max(e, e1, e2x)
            # pack e into pairs on ACT (denominator fused into the first
            # copy); e2's packed innermost dim lets the dgall op hit 2x.
            e2 = small.tile([P, K, 2], F16, tag="e2")
            den = small.tile([P, 1], F32, tag="den")
            nc.scalar.activation(
                e2[:, :, 0:1], e.unsqueeze(2), AF.Copy, accum_out=den
            )
            nc.scalar.copy(e2[:, :, 1:2], e.unsqueeze(2))
            rcp = small.tile([P, 1], F32, tag="rcp")
            nc.vector.reciprocal(rcp, den)

            # all 32 diag blocks in ONE DVE op (2x): idK * e-broadcast
            dgall = dgs.tile([P, K, P], F16, tag="dgall")
            nc.vector.tensor_mul(
                dgall.rearrange("p k (j i) -> p k j i", i=2),
                idK.rearrange("p k (j i) -> p k j i", i=2),
                e2.unsqueeze(2).broadcast_to([P, K, P // 2, 2]),
            )
            o_ps = opsum.tile([P, D], F32, tag="o_ps")
            for k in range(K):
                nc.tensor.matmul(
                    o_ps, dgall[:, k, :], m[:, k, :],
                    start=(k == 0), stop=(k == K - 1), skip_group_check=True,
                )
            o_sb = outs.tile([P, D], F32, tag="o_sb")
            nc.scalar.mul(o_sb, o_ps, rcp[:, 0:1])
            nc.sync.dma_start(out[t * P : (t + 1) * P, :], o_sb)

        # ---- main loop, software-pipelined by one stage: tile t's softmax/
        # aggregation is emitted after tile t+1's phase-1 so the ACT round
        # trip (exp) never bubbles the DVE. Tile 0's phase-1 is emitted
        # before the stc/idK setup work so the DVE starts the moment m0
        # lands; the setup overlaps tile 1's DMA.
        prev = (0, *phase1(0))
        emit_stc()
        emit_idk()
        for t in range(1, nt):
            cur = (t, *phase1(t))
            finish(*prev)
            prev = cur
        finish(*prev)


def build_nc(ns=NS):
    nc = bass.Bass("TRN2", debug=False, num_devices=N_CORES)
    tgt = nc.dram_tensor("target", [ns, D], F32, kind="ExternalInput").ap()
    mid = nc.dram_tensor("middle", [ns, K, D], F32, kind="ExternalInput").ap()
    u_pre = nc.dram_tensor("u_pre", [1, D], F16, kind="ExternalInput").ap()
    c_pre = nc.dram_tensor("c_pre", [1, 1], F32, kind="ExternalInput").ap()
    ident = nc.dram_tensor("ident", [P, P], F16, kind="ExternalInput").ap()
    out = nc.dram_tensor("out", [ns, D], F32, kind="ExternalOutput").ap()
    with tile.TileContext(nc) as tc:
        emit_kernel(tc, out, tgt, mid, u_pre, c_pre, ident, ns)
    import bass_rust as _br

    # Split multi-wait instructions (walrus allows at most 1 sync wait per
    # instruction; Tile can emit more after multi-DMA dependencies).
    _br.generate_event_semaphores(nc)
    return nc


_NC_CACHE = {}


def _get_nc(ns=NS):
    if ns not in _NC_CACHE:
        _NC_CACHE[ns] = build_nc(ns)
    return _NC_CACHE[ns]


def make_in_maps(target, middle, W, b, a_w, a_b):
    target = np.ascontiguousarray(np.asarray(target, dtype=np.float32))
    middle = np.ascontiguousarray(np.asarray(middle, dtype=np.float32))
    W = np.ascontiguousarray(np.asarray(W, dtype=np.float32))
    b = np.ascontiguousarray(np.asarray(b, dtype=np.float32))
    a_w = np.ascontiguousarray(np.asarray(a_w, dtype=np.float32))
    a_b = np.ascontiguousarray(np.asarray(a_b, dtype=np.float32))
    # host-side precompute of the score projection: the W matmuls only
    # enter through the dot with a_w, so u = a_w @ W (a single D-vector)
    # and c = 2*(a_w.b) + a_b cover them exactly.
    u_pre = (a_w @ W).astype(np.float16)            # [1, D]
    c_pre = np.array(
        [[2.0 * float(b @ a_w[0]) + float(a_b[0])]], dtype=np.float32
    )
    ident = np.eye(P, dtype=np.float16)
    tgt_shards = np.split(target, N_CORES, axis=0)
    mid_shards = np.split(middle, N_CORES, axis=0)
    return [
        {
            "target": tgt_shards[i],
            "middle": mid_shards[i],
            "u_pre": u_pre,
            "c_pre": c_pre,
            "ident": ident,
        }
        for i in range(N_CORES)
    ]


def run_sharded(in_maps, **kwargs):
    nc = _get_nc(in_maps[0]["target"].shape[0])
    res = run_bass_kernel_spmd(nc, in_maps, list(range(N_CORES)), **kwargs)
    full = np.concatenate([r["out"] for r in res.results], axis=0)
    return full, res


def kernel(target, middle, W, b, a_w, a_b):
    in_maps = make_in_maps(target, middle, W, b, a_w, a_b)
    full, _ = run_sharded(in_maps)
    return full
